# revision 2
# baseline (speedup 1.0000x reference)
"""Multi-head attention forward (B=4, N=2048, C=1024, H=16) on 8 TRN2 NeuronCores.

Sharding v2: 8 shards = (batch b, head-half hh). Each core projects Q/K/V only
for its 8 heads (512 of 1024 channels) over all 2048 tokens of its batch --
no duplicated K/V work -- then computes attention for those heads over all
tokens. Before the output projection, the two cores of a batch exchange
attention outputs: each core sends y[peer-token-half, its 512 ch] (1MB bf16)
via a pairwise AllGather, overlapped with the attention of its own token half.
A data-driven slot select (mask input) keeps the SPMD graph core-uniform.

Same inner attention machinery as v1: bf16 TensorEngine compute, f32 PSUM,
scores transposed with per-head 64-row PE bands, exp on ScalarE (the critical
engine: 256 x ~1.15us), softmax denominator via ones-column in V, q/k bias
adds moved off ScalarE onto the DVE. Projections woven into the ACT-bound
attention blocks to keep the PE warm.
"""

from contextlib import ExitStack

import numpy as np
import ml_dtypes

import concourse.bass as bass
import concourse.bacc as bacc
import concourse.tile as tile
import concourse.mybir as mybir
from concourse.bass_utils import run_bass_kernel_spmd

F32 = mybir.dt.float32
BF16 = mybir.dt.bfloat16
AF = mybir.ActivationFunctionType
ALU = mybir.AluOpType
BF = ml_dtypes.bfloat16

P = 128
D = 1024
CC = 8          # input-channel 128-blocks (contraction)
CH = 512        # channels (heads) per core
IB = 4          # my channel 128-blocks
NT = 2048       # tokens per batch (all in SBUF; own half first = cols 0:1024)
NO = 1024       # own tokens (output rows)
TB = 16         # 128-token key chunks
KC = 16
DH = 64
SCALE = DH ** -0.5
VS = 8 * 65 + 64  # v slab per tb: 8 heads x (64+den), padded so lhsT [128] stays in-bounds
GROUPS = [[0, 1], [2, 3], [4, 5], [6, 7]]


def bcast_row(nc, out_ap, src_row, n_part):
    """DMA-broadcast one SBUF row [1, N] to [n_part, N] via a 0-step dim."""
    ap0 = src_row.ap[0]
    free = src_row.ap[-1]
    src = bass.AP(src_row.tensor, src_row.offset, [ap0, [0, n_part], free])
    nc.sync.dma_start(out_ap, src)


def attention_body(tc, out, xT, wqT, wkT, wvT, woT, bq, bk, bv, bo, msel,
                   cc_in, cc_out):
    nc = tc.nc
    with ExitStack() as ctx:
        const = ctx.enter_context(tc.tile_pool(name="const", bufs=1))
        qkv = ctx.enter_context(tc.tile_pool(name="qkv", bufs=1))
        xw = ctx.enter_context(tc.tile_pool(name="xw", bufs=1))
        wst = ctx.enter_context(tc.tile_pool(name="wst", bufs=2))
        wib = ctx.enter_context(tc.tile_pool(name="wib", bufs=3))
        # deep e2 buffering: attnV may lag scores/exp by many chunks while the
        # PE drains woven projections -- exp must never block on a free tile
        ee = ctx.enter_context(tc.tile_pool(name="ee", bufs=8))
        rc = ctx.enter_context(tc.tile_pool(name="rc", bufs=1))
        fo = ctx.enter_context(tc.tile_pool(name="fo", bufs=2))
        sp = ctx.enter_context(tc.tile_pool(name="sp", bufs=2, space="PSUM"))
        ao = ctx.enter_context(tc.tile_pool(name="ao", bufs=1, space="PSUM"))
        pj = ctx.enter_context(tc.tile_pool(name="pj", bufs=2, space="PSUM"))

        bq_sb = const.tile([P, IB], F32)
        bk_sb = const.tile([P, IB], F32)
        bv_sb = const.tile([P, IB], F32)
        bo_sb = const.tile([1, D], F32)
        m_sb = const.tile([P, 2], F32)
        nc.scalar.dma_start(bq_sb[:, :], bq[:, :])
        nc.scalar.dma_start(bk_sb[:, :], bk[:, :])
        nc.scalar.dma_start(bv_sb[:, :], bv[:, :])
        nc.scalar.dma_start(bo_sb[:, :], bo[:, :])
        nc.scalar.dma_start(m_sb[:, :], msel[:, :])
        onesf = const.tile([1, P], F32)
        nc.vector.memset(onesf[:, :], 1.0)
        bo_bc = const.tile([P, D], BF16)

        qT_sb = qkv.tile([P, IB * NT], BF16)
        kT_sb = qkv.tile([P, IB * NT], BF16)
        v_sb = qkv.tile([P, TB * VS], BF16)
        yT_sb = qkv.tile([P, IB * NT], BF16)
        yext_sb = qkv.tile([P, IB * NO], BF16)
        sl_sb = qkv.tile([P, 2 * IB * NO], BF16)  # cc_out readback: slot0 | slot1

        # x streams on three queues so the first projection chains unblock fast
        # (ScalarE's queue is idle until the first exp, ~25us in)
        xq = [nc.sync, nc.gpsimd, nc.scalar]
        xT_sb = xw.tile([P, CC * NT], BF16)
        for cc in range(CC):
            xq[cc % 3].dma_start(xT_sb[:, cc * NT:(cc + 1) * NT],
                                 xT[cc * P:(cc + 1) * P, :])

        def load_w(wT_dram, ncols, queues=(nc.sync,)):
            w_sb = wst.tile([P, CC * ncols], BF16, tag="w")
            for cc in range(CC):
                queues[cc % len(queues)].dma_start(
                    w_sb[:, cc * ncols:(cc + 1) * ncols],
                    wT_dram[cc * P:(cc + 1) * P, :])
            return w_sb

        def load_w_ib(wT_dram, ib, queues=(nc.sync, nc.gpsimd)):
            """JIT [1024, 128] column-slice of a weight matrix for one i-block."""
            w_sb = wib.tile([P, CC * P], BF16, tag="wib")
            for cc in range(CC):
                queues[cc % len(queues)].dma_start(
                    w_sb[:, cc * P:(cc + 1) * P],
                    wT_dram[cc * P:(cc + 1) * P, ib * P:(ib + 1) * P])
            return w_sb

        wv_sb = load_w(wvT, CH, (nc.gpsimd,))

        v3 = v_sb.rearrange("p (t s) -> p t s", t=TB)
        nc.vector.memset(v3[:, :, 8 * 65:], 0.0)
        v4 = v3[:, :, 0:8 * 65].rearrange("p t (h c) -> p t h c", c=65)
        nc.vector.memset(v4[:, :, :, 64:65], 1.0)

        def v_proj(tb):
            ps = pj.tile([P, CH], F32, tag="ps")
            for cc in range(CC):
                nc.tensor.matmul(
                    ps[:, :],
                    xT_sb[:, cc * NT + tb * P: cc * NT + (tb + 1) * P],
                    wv_sb[:, cc * CH: (cc + 1) * CH],
                    start=(cc == 0), stop=(cc == CC - 1))
            vsrc = ps.rearrange("p (h c) -> p h c", c=64)
            dst = v_sb[:, tb * VS: tb * VS + 8 * 65].rearrange(
                "p (h c) -> p h c", c=65)[:, :, 0:64]
            nc.vector.tensor_copy(dst, vsrc)

        def q_proj(ib, w_sb, t_order):
            for t in t_order:
                ps = pj.tile([P, 512], F32, tag="ps")
                for cc in range(CC):
                    nc.tensor.matmul(
                        ps[:, :],
                        w_sb[:, cc * P:(cc + 1) * P],
                        xT_sb[:, cc * NT + t * 512: cc * NT + t * 512 + 512],
                        start=(cc == 0), stop=(cc == CC - 1))
                nc.vector.tensor_scalar(
                    qT_sb[:, ib * NT + t * 512: ib * NT + t * 512 + 512],
                    ps[:, :], bq_sb[:, ib:ib + 1], None, op0=ALU.add)

        def k_proj(ib, w_sb, t_order=(0, 1, 2, 3)):
            for t in t_order:
                ps = pj.tile([P, 512], F32, tag="ps")
                for cc in range(CC):
                    nc.tensor.matmul(
                        ps[:, :],
                        w_sb[:, cc * P:(cc + 1) * P],
                        xT_sb[:, cc * NT + t * 512: cc * NT + t * 512 + 512],
                        start=(cc == 0), stop=(cc == CC - 1))
                nc.vector.tensor_scalar(
                    kT_sb[:, ib * NT + t * 512: ib * NT + t * 512 + 512],
                    ps[:, :], bk_sb[:, ib:ib + 1], None, op0=ALU.add)

        # ---- prologue ----
        # Ordered so the first score chunks become ready ASAP: one kT chunk +
        # the first qT chunk, then the rest of kT, then seed v. The scheduler
        # gives earlier-emitted work higher priority, so keep this minimal.
        wq_sl = [None] * IB
        wk0 = load_w_ib(wkT, 0)
        wq_sl[0] = load_w_ib(wqT, 0)
        k_proj(0, wk0, t_order=(0,))
        q_proj(0, wq_sl[0], t_order=(2,))     # pair 0 starts on qb=2
        k_proj(0, wk0, t_order=(1, 2, 3))
        for tb in range(6):
            v_proj(tb)

        # bo broadcast to all partitions (ones matmul, once)
        for chn in range(2):
            ps = pj.tile([P, 512], F32, tag="ps")
            nc.tensor.matmul(ps[:, :], onesf[:, :], bo_sb[:, chn * 512:(chn + 1) * 512],
                             start=True, stop=True)
            nc.vector.tensor_copy(bo_bc[:, chn * 512:(chn + 1) * 512], ps[:, :])

        wo_sb = None

        def out_proj(tb, nch):
            ps = pj.tile([P, 512], F32, tag="ps")
            for j in range(IB):
                nc.tensor.matmul(
                    ps[:, :],
                    yT_sb[:, j * NT + tb * P: j * NT + (tb + 1) * P],
                    wo_sb[:, j * D + nch * 512: j * D + nch * 512 + 512],
                    start=(j == 0), stop=False)
            for j in range(IB):
                nc.tensor.matmul(
                    ps[:, :],
                    yext_sb[:, j * NO + tb * P: j * NO + (tb + 1) * P],
                    wo_sb[:, (IB + j) * D + nch * 512: (IB + j) * D + nch * 512 + 512],
                    start=False, stop=(j == IB - 1))
            os = fo.tile([P, 512], F32, tag="o")
            nc.vector.tensor_tensor(os[:, :], ps[:, :],
                                    bo_bc[:, nch * 512:(nch + 1) * 512], op=ALU.add)
            q = nc.sync if nch == 0 else nc.gpsimd
            q.dma_start(out[tb * P:(tb + 1) * P, nch * 512:(nch + 1) * 512],
                        os[:, :])

        def slot_select(j):
            """yext block j = cc_out[slot0 block j]*m0 + cc_out[slot1 block j]*m1."""
            s0 = sl_sb[:, j * NO:(j + 1) * NO]
            s1 = sl_sb[:, (IB + j) * NO:(IB + j + 1) * NO]
            a = rc.tile([P, NO], F32, tag="sel0")
            b = rc.tile([P, NO], F32, tag="sel1")
            nc.vector.tensor_scalar(a[:, :], s0, m_sb[:, 0:1], None, op0=ALU.mult)
            nc.vector.tensor_scalar(b[:, :], s1, m_sb[:, 1:2], None, op0=ALU.mult)
            nc.vector.tensor_tensor(yext_sb[:, j * NO:(j + 1) * NO], a[:, :], b[:, :],
                                    op=ALU.add)

        # ---- main attention: pr-major (4 query rounds per head pair, peer
        # token half first) so each pair's K/Q weave spreads over the whole
        # previous pair's span instead of piling into round 0 ----
        for pr in range(IB):
            for qi, qb in enumerate((2, 3, 0, 1)):
                qc = qb * 512
                hA, hB = 2 * pr, 2 * pr + 1
                oA = ao.tile([P, 512], F32, tag="oA")
                oB = ao.tile([P, 512], F32, tag="oB")
                vbA = hA * 65
                vbB = hB * 65
                for kc in range(KC):
                    s2 = sp.tile([P, 1024], F32, tag="s")
                    nc.tensor.matmul(
                        s2[:, 0:512],
                        kT_sb[0:64, pr * NT + kc * P: pr * NT + (kc + 1) * P],
                        qT_sb[0:64, pr * NT + qc: pr * NT + qc + 512],
                        start=True, stop=True)
                    nc.tensor.matmul(
                        s2[:, 512:1024],
                        kT_sb[64:128, pr * NT + kc * P: pr * NT + (kc + 1) * P],
                        qT_sb[64:128, pr * NT + qc: pr * NT + qc + 512],
                        start=True, stop=True)
                    e2 = ee.tile([P, 1024], BF16, tag="e")
                    nc.scalar.activation(e2[:, :], s2[:, :], AF.Exp, scale=SCALE)
                    nc.tensor.matmul(
                        oA[:, :],
                        v_sb[:, kc * VS + vbA: kc * VS + vbA + 128],
                        e2[:, 0:512],
                        start=(kc == 0), stop=(kc == KC - 1))
                    nc.tensor.matmul(
                        oB[:, :],
                        v_sb[:, kc * VS + vbB: kc * VS + vbB + 128],
                        e2[:, 512:1024],
                        start=(kc == 0), stop=(kc == KC - 1))
                    # first block: emit remaining v slabs a few chunks ahead
                    # of their attnV consumer (program order defines dataflow)
                    if pr == 0 and qi == 0 and kc < 10:
                        v_proj(6 + kc)
                # evict unnormalized + den rows, then normalize yT in place
                yA = yT_sb[0:64, pr * NT + qc: pr * NT + qc + 512]
                yB = yT_sb[64:128, pr * NT + qc: pr * NT + qc + 512]
                den2 = rc.tile([1, 1024], F32, tag="d")
                last = (pr == IB - 1 and qi == 3)
                nc.vector.tensor_copy(den2[0:1, 0:512], oA[64:65, :])
                nc.vector.tensor_copy(den2[0:1, 512:1024], oB[64:65, :])
                rec2 = rc.tile([1, 1024], F32, tag="rf")
                nc.vector.reciprocal_approx_fast(rec2[0:1, :], den2[0:1, :])
                nc.vector.tensor_copy(yA, oA[0:64, :])
                nc.vector.tensor_copy(yB, oB[0:64, :])
                if not last:
                    bc2 = rc.tile([P, 512], F32, tag="bc")
                    bcast_row(nc, bc2[0:64, :], rec2[0:1, 0:512], 64)
                    bcast_row(nc, bc2[64:128, :], rec2[0:1, 512:1024], 64)
                    bcA = bc2[0:64, :]
                    bcB = bc2[64:128, :]
                else:
                    # tail block: broadcast 1/den with two tiny ones-matmuls so
                    # the final out projection starts the moment yT is final
                    bpA = pj.tile([P, 512], F32, tag="ps")
                    nc.tensor.matmul(bpA[:, :], onesf[:, :], rec2[0:1, 0:512],
                                     start=True, stop=True)
                    bpB = pj.tile([P, 512], F32, tag="ps")
                    nc.tensor.matmul(bpB[:, :], onesf[:, :], rec2[0:1, 512:1024],
                                     start=True, stop=True)
                    bcA = bpA[0:64, :]
                    bcB = bpB[64:128, :]
                nc.vector.tensor_tensor(yA, yA, bcA, op=ALU.mult)
                nc.vector.tensor_scalar(yA, yA, bv_sb[0:64, pr:pr + 1], None, op0=ALU.add)
                nc.vector.tensor_tensor(yB, yB, bcB, op=ALU.mult)
                nc.vector.tensor_scalar(yB, yB, bv_sb[64:128, pr:pr + 1], None, op0=ALU.add)

                # ---- woven work (ordered by deadline, emitted low-priority
                # after each block so it fills the ACT-bound PE gaps) ----
                if pr == 0:
                    if qi == 0:
                        q_proj(0, wq_sl[0], t_order=(3, 0))
                    elif qi == 1:
                        q_proj(0, wq_sl[0], t_order=(1,))
                else:
                    if qi == 0:
                        q_proj(pr, wq_sl[pr], t_order=(0,))
                    elif qi == 1:
                        q_proj(pr, wq_sl[pr], t_order=(1,))
                if qi == 2 and pr + 1 < IB:
                    wk_next = load_w_ib(wkT, pr + 1)
                    wq_sl[pr + 1] = load_w_ib(wqT, pr + 1)
                    k_proj(pr + 1, wk_next)
                    q_proj(pr + 1, wq_sl[pr + 1], t_order=(2,))
                if qi == 3 and pr + 1 < IB:
                    q_proj(pr + 1, wq_sl[pr + 1], t_order=(3,))
                if pr == 1 and qi == 0:
                    wo_sb = load_w(woT, D, (nc.sync, nc.gpsimd))
                if pr == IB - 1 and qi == 1:
                    # all peer-token y complete: ship it and fire the exchange
                    for j in range(IB):
                        nc.sync.dma_start(
                            cc_in[j * P:(j + 1) * P, :],
                            yT_sb[:, j * NT + NO: j * NT + NT])
                    nc.gpsimd.collective_compute(
                        "AllGather",
                        ALU.bypass,
                        replica_groups=GROUPS,
                        ins=[cc_in[:, :]],
                        outs=[cc_out[:, :]],
                    )
                if pr == IB - 1 and qi == 2:
                    # pull both AG slots as they land; select peer's half
                    for j in range(IB):
                        nc.gpsimd.dma_start(
                            sl_sb[:, j * NO:(j + 1) * NO],
                            cc_out[j * P:(j + 1) * P, :])
                        nc.gpsimd.dma_start(
                            sl_sb[:, (IB + j) * NO:(IB + j + 1) * NO],
                            cc_out[CH + j * P: CH + (j + 1) * P, :])
                    for j in range(IB):
                        slot_select(j)
                if pr == IB - 1 and qi == 3:
                    # token rows 0:512 (qb=0, previous round) are final --
                    # weave their output projection under the last exp span
                    for tb in range(4):
                        out_proj(tb, 0)
                        out_proj(tb, 1)

        # ---- tail: output projection for the last token rows ----
        for tb in range(4, 8):
            out_proj(tb, 0)
            out_proj(tb, 1)


N_CORES = 8

_GRAPH_CACHE = {}


def build_graph():
    if "nc" in _GRAPH_CACHE:
        return _GRAPH_CACHE["nc"]
    nc = bacc.Bacc("TRN2", target_bir_lowering=False, debug=False,
                   num_devices=N_CORES)
    xT = nc.dram_tensor("xT", [D, NT], BF16, kind="ExternalInput").ap()
    wqT = nc.dram_tensor("wqT", [D, CH], BF16, kind="ExternalInput").ap()
    wkT = nc.dram_tensor("wkT", [D, CH], BF16, kind="ExternalInput").ap()
    wvT = nc.dram_tensor("wvT", [D, CH], BF16, kind="ExternalInput").ap()
    woT = nc.dram_tensor("woT", [D, D], BF16, kind="ExternalInput").ap()
    bq = nc.dram_tensor("bq", [P, IB], F32, kind="ExternalInput").ap()
    bk = nc.dram_tensor("bk", [P, IB], F32, kind="ExternalInput").ap()
    bv = nc.dram_tensor("bv", [P, IB], F32, kind="ExternalInput").ap()
    bo = nc.dram_tensor("bo", [1, D], F32, kind="ExternalInput").ap()
    msel = nc.dram_tensor("msel", [P, 2], F32, kind="ExternalInput").ap()
    out = nc.dram_tensor("out", [NO, D], F32, kind="ExternalOutput").ap()
    cc_in = nc.dram_tensor("cc_in", [CH, NO], BF16).ap()
    cc_out = nc.dram_tensor("cc_out", [2 * CH, NO], BF16).ap()
    with tile.TileContext(nc) as tc:
        attention_body(tc, out, xT, wqT, wkT, wvT, woT, bq, bk, bv, bo, msel,
                       cc_in, cc_out)
    nc.compile()
    _GRAPH_CACHE["nc"] = nc
    return nc


def make_in_maps(x, Wq, bq, Wk, bk, Wv, bv, Wo, bo):
    x = np.asarray(x, np.float32)
    wqTf = np.ascontiguousarray(np.asarray(Wq, np.float32).T)
    wkTf = np.ascontiguousarray(np.asarray(Wk, np.float32).T)
    wvTf = np.ascontiguousarray(np.asarray(Wv, np.float32).T)
    woTf = np.ascontiguousarray(np.asarray(Wo, np.float32).T)
    bqf = np.asarray(bq, np.float32)
    bkf = np.asarray(bk, np.float32)
    bvf = np.asarray(bv, np.float32)
    bof = np.asarray(bo, np.float32).reshape(1, D)
    in_maps = []
    for core in range(N_CORES):
        b, hh = core // 2, core % 2
        xb = x[b]
        if hh == 1:
            xb = np.concatenate([xb[NO:], xb[:NO]], axis=0)
        sl = slice(hh * CH, (hh + 1) * CH)
        pe = slice((1 - hh) * CH, (2 - hh) * CH)
        mvec = np.zeros((P, 2), np.float32)
        mvec[:, 1 - hh] = 1.0   # even reads slot1 (odd's contribution)
        in_maps.append({
            "xT": np.ascontiguousarray(xb.T).astype(BF),
            "wqT": np.ascontiguousarray(wqTf[:, sl]).astype(BF),
            "wkT": np.ascontiguousarray(wkTf[:, sl]).astype(BF),
            "wvT": np.ascontiguousarray(wvTf[:, sl]).astype(BF),
            "woT": np.ascontiguousarray(
                np.concatenate([woTf[sl, :], woTf[pe, :]], axis=0)).astype(BF),
            "bq": np.ascontiguousarray(bqf[sl].reshape(IB, P).T),
            "bk": np.ascontiguousarray(bkf[sl].reshape(IB, P).T),
            "bv": np.ascontiguousarray(bvf[sl].reshape(IB, P).T),
            "bo": bof,
            "msel": mvec,
        })
    return in_maps


def run(inputs, trace=False, **kw):
    nc = build_graph()
    in_maps = make_in_maps(**inputs)
    res = run_bass_kernel_spmd(nc, in_maps, list(range(N_CORES)), trace=trace, **kw)
    x = np.asarray(inputs["x"], np.float32)
    B, N, C = x.shape
    out = np.empty((B, N, C), np.float32)
    for core in range(N_CORES):
        b, hh = core // 2, core % 2
        out[b, hh * NO:(hh + 1) * NO, :] = res.results[core]["out"]
    return out, res


def kernel(x, Wq, bq, Wk, bk, Wv, bv, Wo, bo):
    out, _ = run(dict(x=x, Wq=Wq, bq=bq, Wk=Wk, bk=bk, Wv=Wv, bv=bv, Wo=Wo, bo=bo))
    return out


# revision 3
# speedup vs baseline: 1.0369x; 1.0369x over previous
"""Multi-head attention forward (B=4, N=2048, C=1024, H=16) on 8 TRN2 NeuronCores.

Sharding v2: 8 shards = (batch b, head-half hh). Each core projects Q/K/V only
for its 8 heads (512 of 1024 channels) over all 2048 tokens of its batch --
no duplicated K/V work -- then computes attention for those heads over all
tokens. Before the output projection, the two cores of a batch exchange
attention outputs: each core sends y[peer-token-half, its 512 ch] (1MB bf16)
via a pairwise AllGather, overlapped with the attention of its own token half.
A data-driven slot select (mask input) keeps the SPMD graph core-uniform.

Same inner attention machinery as v1: bf16 TensorEngine compute, f32 PSUM,
scores transposed with per-head 64-row PE bands, exp on ScalarE (the critical
engine: 256 x ~1.15us), softmax denominator via ones-column in V, q/k bias
adds moved off ScalarE onto the DVE. Projections woven into the ACT-bound
attention blocks to keep the PE warm.
"""

from contextlib import ExitStack

import numpy as np
import ml_dtypes

import concourse.bass as bass
import concourse.bacc as bacc
import concourse.tile as tile
import concourse.mybir as mybir
from concourse.bass_utils import run_bass_kernel_spmd

F32 = mybir.dt.float32
BF16 = mybir.dt.bfloat16
AF = mybir.ActivationFunctionType
ALU = mybir.AluOpType
BF = ml_dtypes.bfloat16

P = 128
D = 1024
CC = 8          # input-channel 128-blocks (contraction)
CH = 512        # channels (heads) per core
IB = 4          # my channel 128-blocks
NT = 2048       # tokens per batch (all in SBUF; own half first = cols 0:1024)
NO = 1024       # own tokens (output rows)
TB = 16         # 128-token key chunks
KC = 16
DH = 64
SCALE = DH ** -0.5
VS = 8 * 65 + 64  # v slab per tb: 8 heads x (64+den), padded so lhsT [128] stays in-bounds
GROUPS = [[0, 1], [2, 3], [4, 5], [6, 7]]


def bcast_row(nc, out_ap, src_row, n_part):
    """DMA-broadcast one SBUF row [1, N] to [n_part, N] via a 0-step dim."""
    ap0 = src_row.ap[0]
    free = src_row.ap[-1]
    src = bass.AP(src_row.tensor, src_row.offset, [ap0, [0, n_part], free])
    nc.sync.dma_start(out_ap, src)


def attention_body(tc, out, xT, wqT, wkT, wvT, woT, bq, bk, bv, bo, msel,
                   cc_in, cc_out):
    nc = tc.nc
    with ExitStack() as ctx:
        const = ctx.enter_context(tc.tile_pool(name="const", bufs=1))
        qkv = ctx.enter_context(tc.tile_pool(name="qkv", bufs=1))
        xw = ctx.enter_context(tc.tile_pool(name="xw", bufs=1))
        wst = ctx.enter_context(tc.tile_pool(name="wst", bufs=2))
        wib = ctx.enter_context(tc.tile_pool(name="wib", bufs=3))
        # deep e2 buffering: attnV may lag scores/exp by many chunks while the
        # PE drains woven projections -- exp must never block on a free tile
        ee = ctx.enter_context(tc.tile_pool(name="ee", bufs=8))
        rc = ctx.enter_context(tc.tile_pool(name="rc", bufs=1))
        fo = ctx.enter_context(tc.tile_pool(name="fo", bufs=2))
        sp = ctx.enter_context(tc.tile_pool(name="sp", bufs=2, space="PSUM"))
        ao = ctx.enter_context(tc.tile_pool(name="ao", bufs=1, space="PSUM"))
        pj = ctx.enter_context(tc.tile_pool(name="pj", bufs=2, space="PSUM"))

        bq_sb = const.tile([P, IB], F32)
        bk_sb = const.tile([P, IB], F32)
        bv_sb = const.tile([P, IB], F32)
        bo_sb = const.tile([1, D], F32)
        m_sb = const.tile([P, 2], F32)
        nc.scalar.dma_start(bq_sb[:, :], bq[:, :])
        nc.scalar.dma_start(bk_sb[:, :], bk[:, :])
        nc.scalar.dma_start(bv_sb[:, :], bv[:, :])
        nc.scalar.dma_start(bo_sb[:, :], bo[:, :])
        nc.scalar.dma_start(m_sb[:, :], msel[:, :])
        onesf = const.tile([1, P], F32)
        nc.vector.memset(onesf[:, :], 1.0)
        bo_bc = const.tile([P, D], BF16)

        qT_sb = qkv.tile([P, IB * NT], BF16)
        kT_sb = qkv.tile([P, IB * NT], BF16)
        v_sb = qkv.tile([P, TB * VS], BF16)
        yT_sb = qkv.tile([P, IB * NT], BF16)
        yext_sb = qkv.tile([P, IB * NO], BF16)
        sl_sb = qkv.tile([P, 2 * IB * NO], BF16)  # cc_out readback: slot0 | slot1

        # x streams on three queues so the first projection chains unblock fast
        # (ScalarE's queue is idle until the first exp, ~25us in)
        xq = [nc.sync, nc.gpsimd, nc.scalar]
        xT_sb = xw.tile([P, CC * NT], BF16)
        for cc in range(CC):
            xq[cc % 3].dma_start(xT_sb[:, cc * NT:(cc + 1) * NT],
                                 xT[cc * P:(cc + 1) * P, :])

        def load_w(wT_dram, ncols, queues=(nc.sync,)):
            w_sb = wst.tile([P, CC * ncols], BF16, tag="w")
            for cc in range(CC):
                queues[cc % len(queues)].dma_start(
                    w_sb[:, cc * ncols:(cc + 1) * ncols],
                    wT_dram[cc * P:(cc + 1) * P, :])
            return w_sb

        def load_w_ib(wT_dram, ib, queues=(nc.sync, nc.gpsimd)):
            """JIT [1024, 128] column-slice of a weight matrix for one i-block."""
            w_sb = wib.tile([P, CC * P], BF16, tag="wib")
            for cc in range(CC):
                queues[cc % len(queues)].dma_start(
                    w_sb[:, cc * P:(cc + 1) * P],
                    wT_dram[cc * P:(cc + 1) * P, ib * P:(ib + 1) * P])
            return w_sb

        wv_sb = load_w(wvT, CH, (nc.sync, nc.gpsimd))

        v3 = v_sb.rearrange("p (t s) -> p t s", t=TB)
        nc.vector.memset(v3[:, :, 8 * 65:], 0.0)
        v4 = v3[:, :, 0:8 * 65].rearrange("p t (h c) -> p t h c", c=65)
        nc.vector.memset(v4[:, :, :, 64:65], 1.0)

        def v_proj(tb):
            ps = pj.tile([P, CH], F32, tag="ps")
            for cc in range(CC):
                nc.tensor.matmul(
                    ps[:, :],
                    xT_sb[:, cc * NT + tb * P: cc * NT + (tb + 1) * P],
                    wv_sb[:, cc * CH: (cc + 1) * CH],
                    start=(cc == 0), stop=(cc == CC - 1))
            vsrc = ps.rearrange("p (h c) -> p h c", c=64)
            dst = v_sb[:, tb * VS: tb * VS + 8 * 65].rearrange(
                "p (h c) -> p h c", c=65)[:, :, 0:64]
            nc.vector.tensor_copy(dst, vsrc)

        def q_proj(ib, w_sb, t_order):
            for t in t_order:
                ps = pj.tile([P, 512], F32, tag="ps")
                for cc in range(CC):
                    nc.tensor.matmul(
                        ps[:, :],
                        w_sb[:, cc * P:(cc + 1) * P],
                        xT_sb[:, cc * NT + t * 512: cc * NT + t * 512 + 512],
                        start=(cc == 0), stop=(cc == CC - 1))
                nc.vector.tensor_scalar(
                    qT_sb[:, ib * NT + t * 512: ib * NT + t * 512 + 512],
                    ps[:, :], bq_sb[:, ib:ib + 1], None, op0=ALU.add)

        def k_proj(ib, w_sb, t_order=(0, 1, 2, 3)):
            for t in t_order:
                ps = pj.tile([P, 512], F32, tag="ps")
                for cc in range(CC):
                    nc.tensor.matmul(
                        ps[:, :],
                        w_sb[:, cc * P:(cc + 1) * P],
                        xT_sb[:, cc * NT + t * 512: cc * NT + t * 512 + 512],
                        start=(cc == 0), stop=(cc == CC - 1))
                nc.vector.tensor_scalar(
                    kT_sb[:, ib * NT + t * 512: ib * NT + t * 512 + 512],
                    ps[:, :], bk_sb[:, ib:ib + 1], None, op0=ALU.add)

        # ---- prologue ----
        # Ordered so the first score chunks become ready ASAP: one kT chunk +
        # the first qT chunk, then the rest of kT, then seed v. The scheduler
        # gives earlier-emitted work higher priority, so keep this minimal.
        wq_sl = [None] * IB
        wk0 = load_w_ib(wkT, 0)
        wq_sl[0] = load_w_ib(wqT, 0)
        k_proj(0, wk0, t_order=(0,))
        q_proj(0, wq_sl[0], t_order=(2,))     # pair 0 starts on qb=2
        k_proj(0, wk0, t_order=(1, 2, 3))
        for tb in range(6):
            v_proj(tb)

        # bo broadcast to all partitions (ones matmul, once)
        for chn in range(2):
            ps = pj.tile([P, 512], F32, tag="ps")
            nc.tensor.matmul(ps[:, :], onesf[:, :], bo_sb[:, chn * 512:(chn + 1) * 512],
                             start=True, stop=True)
            nc.vector.tensor_copy(bo_bc[:, chn * 512:(chn + 1) * 512], ps[:, :])

        wo_sb = None

        def out_proj(tb, nch):
            ps = pj.tile([P, 512], F32, tag="ps")
            for j in range(IB):
                nc.tensor.matmul(
                    ps[:, :],
                    yT_sb[:, j * NT + tb * P: j * NT + (tb + 1) * P],
                    wo_sb[:, j * D + nch * 512: j * D + nch * 512 + 512],
                    start=(j == 0), stop=False)
            for j in range(IB):
                nc.tensor.matmul(
                    ps[:, :],
                    yext_sb[:, j * NO + tb * P: j * NO + (tb + 1) * P],
                    wo_sb[:, (IB + j) * D + nch * 512: (IB + j) * D + nch * 512 + 512],
                    start=False, stop=(j == IB - 1))
            os = fo.tile([P, 512], F32, tag="o")
            nc.vector.tensor_tensor(os[:, :], ps[:, :],
                                    bo_bc[:, nch * 512:(nch + 1) * 512], op=ALU.add)
            q = nc.sync if nch == 0 else nc.gpsimd
            q.dma_start(out[tb * P:(tb + 1) * P, nch * 512:(nch + 1) * 512],
                        os[:, :])

        def slot_pull(j):
            """DMA pair-j's AG output slots into SBUF and select peer half."""
            nc.gpsimd.dma_start(
                sl_sb[:, j * NO:(j + 1) * NO],
                cc_out[2 * j * P:(2 * j + 1) * P, :])
            nc.gpsimd.dma_start(
                sl_sb[:, (IB + j) * NO:(IB + j + 1) * NO],
                cc_out[(2 * j + 1) * P:(2 * j + 2) * P, :])
            slot_select(j)

        def slot_select(j):
            """yext block j = cc_out[slot0 block j]*m0 + cc_out[slot1 block j]*m1."""
            s0 = sl_sb[:, j * NO:(j + 1) * NO]
            s1 = sl_sb[:, (IB + j) * NO:(IB + j + 1) * NO]
            a = rc.tile([P, NO], BF16, tag="sel0")
            b = rc.tile([P, NO], BF16, tag="sel1")
            nc.vector.tensor_scalar(a[:, :], s0, m_sb[:, 0:1], None, op0=ALU.mult)
            nc.vector.tensor_scalar(b[:, :], s1, m_sb[:, 1:2], None, op0=ALU.mult)
            nc.vector.tensor_tensor(yext_sb[:, j * NO:(j + 1) * NO], a[:, :], b[:, :],
                                    op=ALU.add)

        # ---- main attention: pr-major (4 query rounds per head pair, peer
        # token half first) so each pair's K/Q weave spreads over the whole
        # previous pair's span instead of piling into round 0 ----
        for pr in range(IB):
            for qi, qb in enumerate((2, 3, 0, 1)):
                qc = qb * 512
                hA, hB = 2 * pr, 2 * pr + 1
                oA = ao.tile([P, 512], F32, tag="oA")
                oB = ao.tile([P, 512], F32, tag="oB")
                vbA = hA * 65
                vbB = hB * 65
                for kc in range(KC):
                    s2 = sp.tile([P, 1024], F32, tag="s")
                    nc.tensor.matmul(
                        s2[:, 0:512],
                        kT_sb[0:64, pr * NT + kc * P: pr * NT + (kc + 1) * P],
                        qT_sb[0:64, pr * NT + qc: pr * NT + qc + 512],
                        start=True, stop=True)
                    nc.tensor.matmul(
                        s2[:, 512:1024],
                        kT_sb[64:128, pr * NT + kc * P: pr * NT + (kc + 1) * P],
                        qT_sb[64:128, pr * NT + qc: pr * NT + qc + 512],
                        start=True, stop=True)
                    e2 = ee.tile([P, 1024], BF16, tag="e")
                    nc.scalar.activation(e2[:, :], s2[:, :], AF.Exp, scale=SCALE)
                    nc.tensor.matmul(
                        oA[:, :],
                        v_sb[:, kc * VS + vbA: kc * VS + vbA + 128],
                        e2[:, 0:512],
                        start=(kc == 0), stop=(kc == KC - 1))
                    nc.tensor.matmul(
                        oB[:, :],
                        v_sb[:, kc * VS + vbB: kc * VS + vbB + 128],
                        e2[:, 512:1024],
                        start=(kc == 0), stop=(kc == KC - 1))
                    # first block: emit remaining v slabs a few chunks ahead
                    # of their attnV consumer (program order defines dataflow)
                    if pr == 0 and qi == 0 and kc < 10:
                        v_proj(6 + kc)
                # evict unnormalized + den rows, then normalize yT in place
                yA = yT_sb[0:64, pr * NT + qc: pr * NT + qc + 512]
                yB = yT_sb[64:128, pr * NT + qc: pr * NT + qc + 512]
                den2 = rc.tile([1, 1024], F32, tag="d")
                last = (pr == IB - 1 and qi == 3)
                nc.vector.tensor_copy(den2[0:1, 0:512], oA[64:65, :])
                nc.vector.tensor_copy(den2[0:1, 512:1024], oB[64:65, :])
                rec2 = rc.tile([1, 1024], F32, tag="rf")
                nc.vector.reciprocal_approx_fast(rec2[0:1, :], den2[0:1, :])
                nc.vector.tensor_copy(yA, oA[0:64, :])
                nc.vector.tensor_copy(yB, oB[0:64, :])
                if not last:
                    bc2 = rc.tile([P, 512], F32, tag="bc")
                    bcast_row(nc, bc2[0:64, :], rec2[0:1, 0:512], 64)
                    bcast_row(nc, bc2[64:128, :], rec2[0:1, 512:1024], 64)
                    bcA = bc2[0:64, :]
                    bcB = bc2[64:128, :]
                else:
                    # tail block: broadcast 1/den with two tiny ones-matmuls so
                    # the final out projection starts the moment yT is final
                    bpA = pj.tile([P, 512], F32, tag="ps")
                    nc.tensor.matmul(bpA[:, :], onesf[:, :], rec2[0:1, 0:512],
                                     start=True, stop=True)
                    bpB = pj.tile([P, 512], F32, tag="ps")
                    nc.tensor.matmul(bpB[:, :], onesf[:, :], rec2[0:1, 512:1024],
                                     start=True, stop=True)
                    bcA = bpA[0:64, :]
                    bcB = bpB[64:128, :]
                nc.vector.tensor_tensor(yA, yA, bcA, op=ALU.mult)
                nc.vector.tensor_scalar(yA, yA, bv_sb[0:64, pr:pr + 1], None, op0=ALU.add)
                nc.vector.tensor_tensor(yB, yB, bcB, op=ALU.mult)
                nc.vector.tensor_scalar(yB, yB, bv_sb[64:128, pr:pr + 1], None, op0=ALU.add)

                # ---- woven work (ordered by deadline, emitted low-priority
                # after each block so it fills the ACT-bound PE gaps) ----
                if pr == 0:
                    if qi == 0:
                        q_proj(0, wq_sl[0], t_order=(3, 0))
                    elif qi == 1:
                        q_proj(0, wq_sl[0], t_order=(1,))
                else:
                    if qi == 0:
                        q_proj(pr, wq_sl[pr], t_order=(0,))
                    elif qi == 1:
                        q_proj(pr, wq_sl[pr], t_order=(1,))
                if qi == 2 and pr + 1 < IB:
                    wk_next = load_w_ib(wkT, pr + 1)
                    wq_sl[pr + 1] = load_w_ib(wqT, pr + 1)
                    k_proj(pr + 1, wk_next)
                    q_proj(pr + 1, wq_sl[pr + 1], t_order=(2,))
                if qi == 3 and pr + 1 < IB:
                    q_proj(pr + 1, wq_sl[pr + 1], t_order=(3,))
                if pr == 1 and qi == 0:
                    wo_sb = load_w(woT, D, (nc.sync,))  # keep gpsimd free for the AGs
                if qi == 1:
                    # this pair's peer-token y is complete: fire its own
                    # small AllGather immediately (only the last pair's
                    # 256KB exchange lands near the tail)
                    nc.sync.dma_start(
                        cc_in[pr * P:(pr + 1) * P, :],
                        yT_sb[:, pr * NT + NO: pr * NT + NT])
                    nc.gpsimd.collective_compute(
                        "AllGather",
                        ALU.bypass,
                        replica_groups=GROUPS,
                        ins=[cc_in[pr * P:(pr + 1) * P, :]],
                        outs=[cc_out[2 * pr * P:2 * (pr + 1) * P, :]],
                    )

                if pr == IB - 1 and qi == 0:
                    # pairs 0-2 have long since landed: one pull burst here,
                    # clear of the per-block normalize chains
                    slot_pull(0)
                    slot_pull(1)
                    slot_pull(2)
                if pr == IB - 1 and qi == 2:
                    slot_pull(IB - 1)
                if pr == IB - 1 and qi == 3:
                    # token rows 0:512 (qb=0, previous round) are final --
                    # weave their output projection under the last exp span
                    for tb in range(4):
                        out_proj(tb, 0)
                        out_proj(tb, 1)

        # ---- tail: output projection for the last token rows ----
        for tb in range(4, 8):
            out_proj(tb, 0)
            out_proj(tb, 1)


N_CORES = 8

_GRAPH_CACHE = {}


def build_graph():
    if "nc" in _GRAPH_CACHE:
        return _GRAPH_CACHE["nc"]
    nc = bacc.Bacc("TRN2", target_bir_lowering=False, debug=False,
                   num_devices=N_CORES)
    xT = nc.dram_tensor("xT", [D, NT], BF16, kind="ExternalInput").ap()
    wqT = nc.dram_tensor("wqT", [D, CH], BF16, kind="ExternalInput").ap()
    wkT = nc.dram_tensor("wkT", [D, CH], BF16, kind="ExternalInput").ap()
    wvT = nc.dram_tensor("wvT", [D, CH], BF16, kind="ExternalInput").ap()
    woT = nc.dram_tensor("woT", [D, D], BF16, kind="ExternalInput").ap()
    bq = nc.dram_tensor("bq", [P, IB], F32, kind="ExternalInput").ap()
    bk = nc.dram_tensor("bk", [P, IB], F32, kind="ExternalInput").ap()
    bv = nc.dram_tensor("bv", [P, IB], F32, kind="ExternalInput").ap()
    bo = nc.dram_tensor("bo", [1, D], F32, kind="ExternalInput").ap()
    msel = nc.dram_tensor("msel", [P, 2], F32, kind="ExternalInput").ap()
    out = nc.dram_tensor("out", [NO, D], F32, kind="ExternalOutput").ap()
    cc_in = nc.dram_tensor("cc_in", [CH, NO], BF16).ap()
    cc_out = nc.dram_tensor("cc_out", [2 * CH, NO], BF16).ap()
    with tile.TileContext(nc) as tc:
        attention_body(tc, out, xT, wqT, wkT, wvT, woT, bq, bk, bv, bo, msel,
                       cc_in, cc_out)
    nc.compile()
    _GRAPH_CACHE["nc"] = nc
    return nc


def make_in_maps(x, Wq, bq, Wk, bk, Wv, bv, Wo, bo):
    x = np.asarray(x, np.float32)
    wqTf = np.ascontiguousarray(np.asarray(Wq, np.float32).T)
    wkTf = np.ascontiguousarray(np.asarray(Wk, np.float32).T)
    wvTf = np.ascontiguousarray(np.asarray(Wv, np.float32).T)
    woTf = np.ascontiguousarray(np.asarray(Wo, np.float32).T)
    bqf = np.asarray(bq, np.float32)
    bkf = np.asarray(bk, np.float32)
    bvf = np.asarray(bv, np.float32)
    bof = np.asarray(bo, np.float32).reshape(1, D)
    in_maps = []
    for core in range(N_CORES):
        b, hh = core // 2, core % 2
        xb = x[b]
        if hh == 1:
            xb = np.concatenate([xb[NO:], xb[:NO]], axis=0)
        sl = slice(hh * CH, (hh + 1) * CH)
        pe = slice((1 - hh) * CH, (2 - hh) * CH)
        mvec = np.zeros((P, 2), np.float32)
        mvec[:, 1 - hh] = 1.0   # even reads slot1 (odd's contribution)
        in_maps.append({
            "xT": np.ascontiguousarray(xb.T).astype(BF),
            "wqT": np.ascontiguousarray(wqTf[:, sl]).astype(BF),
            "wkT": np.ascontiguousarray(wkTf[:, sl]).astype(BF),
            "wvT": np.ascontiguousarray(wvTf[:, sl]).astype(BF),
            "woT": np.ascontiguousarray(
                np.concatenate([woTf[sl, :], woTf[pe, :]], axis=0)).astype(BF),
            "bq": np.ascontiguousarray(bqf[sl].reshape(IB, P).T),
            "bk": np.ascontiguousarray(bkf[sl].reshape(IB, P).T),
            "bv": np.ascontiguousarray(bvf[sl].reshape(IB, P).T),
            "bo": bof,
            "msel": mvec,
        })
    return in_maps


def run(inputs, trace=False, **kw):
    nc = build_graph()
    in_maps = make_in_maps(**inputs)
    res = run_bass_kernel_spmd(nc, in_maps, list(range(N_CORES)), trace=trace, **kw)
    x = np.asarray(inputs["x"], np.float32)
    B, N, C = x.shape
    out = np.empty((B, N, C), np.float32)
    for core in range(N_CORES):
        b, hh = core // 2, core % 2
        out[b, hh * NO:(hh + 1) * NO, :] = res.results[core]["out"]
    return out, res


def kernel(x, Wq, bq, Wk, bk, Wv, bv, Wo, bo):
    out, _ = run(dict(x=x, Wq=Wq, bq=bq, Wk=Wk, bk=bk, Wv=Wv, bv=bv, Wo=Wo, bo=bo))
    return out


# revision 4
# speedup vs baseline: 1.2162x; 1.1730x over previous
"""Multi-head attention forward (B=4, N=2048, C=1024, H=16) on 8 TRN2 NeuronCores.

Sharding v2: 8 shards = (batch b, head-half hh). Each core projects Q/K/V only
for its 8 heads (512 of 1024 channels) over all 2048 tokens of its batch --
no duplicated K/V work -- then computes attention for those heads over all
tokens. Before the output projection, the two cores of a batch exchange
attention outputs: each core sends y[peer-token-half, its 512 ch] (1MB bf16)
via a pairwise AllGather, overlapped with the attention of its own token half.
A data-driven slot select (mask input) keeps the SPMD graph core-uniform.

Same inner attention machinery as v1: bf16 TensorEngine compute, f32 PSUM,
scores transposed with per-head 64-row PE bands, exp on ScalarE (the critical
engine: 256 x ~1.15us), softmax denominator via ones-column in V, q/k bias
adds moved off ScalarE onto the DVE. Projections woven into the ACT-bound
attention blocks to keep the PE warm.
"""

from contextlib import ExitStack

import numpy as np
import ml_dtypes

import concourse.bass as bass
import concourse.bacc as bacc
import concourse.tile as tile
import concourse.mybir as mybir
from concourse.bass_utils import run_bass_kernel_spmd

F32 = mybir.dt.float32
BF16 = mybir.dt.bfloat16
AF = mybir.ActivationFunctionType
ALU = mybir.AluOpType
BF = ml_dtypes.bfloat16
F8 = mybir.dt.float8e4
BF8 = ml_dtypes.float8_e4m3

P = 128
D = 1024
CC = 8          # input-channel 128-blocks (contraction)
CH = 512        # channels (heads) per core
IB = 4          # my channel 128-blocks
NT = 2048       # tokens per batch (all in SBUF; own half first = cols 0:1024)
NO = 1024       # own tokens (output rows)
TB = 16         # 128-token key chunks
KC = 16
DH = 64
SCALE = DH ** -0.5
VS = 8 * 65 + 64  # v slab per tb: 8 heads x (64+den), padded so lhsT [128] stays in-bounds
GROUPS = [[0, 1], [2, 3], [4, 5], [6, 7]]


def bcast_row(nc, out_ap, src_row, n_part):
    """DMA-broadcast one SBUF row [1, N] to [n_part, N] via a 0-step dim."""
    ap0 = src_row.ap[0]
    free = src_row.ap[-1]
    src = bass.AP(src_row.tensor, src_row.offset, [ap0, [0, n_part], free])
    nc.sync.dma_start(out_ap, src)


def attention_body(tc, out, xT, wqT, wkT, wvT, woT, bq, bk, bv, bo, msel,
                   cc_in, cc_out):
    nc = tc.nc
    with ExitStack() as ctx:
        const = ctx.enter_context(tc.tile_pool(name="const", bufs=1))
        qkv = ctx.enter_context(tc.tile_pool(name="qkv", bufs=1))
        xw = ctx.enter_context(tc.tile_pool(name="xw", bufs=1))
        wst = ctx.enter_context(tc.tile_pool(name="wst", bufs=2))
        wib = ctx.enter_context(tc.tile_pool(name="wib", bufs=3))
        # deep e2 buffering: attnV may lag scores/exp by many chunks while the
        # PE drains woven projections -- exp must never block on a free tile
        ee = ctx.enter_context(tc.tile_pool(name="ee", bufs=8))
        rc = ctx.enter_context(tc.tile_pool(name="rc", bufs=1))
        fo = ctx.enter_context(tc.tile_pool(name="fo", bufs=2))
        sp = ctx.enter_context(tc.tile_pool(name="sp", bufs=2, space="PSUM"))
        ao = ctx.enter_context(tc.tile_pool(name="ao", bufs=1, space="PSUM"))
        pj = ctx.enter_context(tc.tile_pool(name="pj", bufs=2, space="PSUM"))

        bq_sb = const.tile([P, IB], F32)
        bk_sb = const.tile([P, IB], F32)
        bv_sb = const.tile([P, IB], F32)
        bo_sb = const.tile([1, D], F32)
        m_sb = const.tile([P, 2], F32)
        nc.scalar.dma_start(bq_sb[:, :], bq[:, :])
        nc.scalar.dma_start(bk_sb[:, :], bk[:, :])
        nc.scalar.dma_start(bv_sb[:, :], bv[:, :])
        nc.scalar.dma_start(bo_sb[:, :], bo[:, :])
        nc.scalar.dma_start(m_sb[:, :], msel[:, :])
        onesf = const.tile([1, P], F32)
        nc.vector.memset(onesf[:, :], 1.0)
        bo_bc = const.tile([P, D], BF16)

        qT_sb = qkv.tile([P, IB * NT], BF16)
        kT_sb = qkv.tile([P, IB * NT], BF16)
        v_sb = qkv.tile([P, TB * VS], BF16)
        yT_sb = qkv.tile([P, IB * NT], BF16)
        yext_sb = qkv.tile([P, IB * NO], BF16)

        # x streams on three queues so the first projection chains unblock fast
        # (ScalarE's queue is idle until the first exp, ~25us in)
        xq = [nc.sync, nc.gpsimd, nc.scalar]
        xT_sb = xw.tile([P, CC * NT], BF16)
        x8_sb = xw.tile([P, CC * NT], F8)   # fp8 shadow of x for the V matmuls
        for cc in range(CC):
            xq[cc % 3].dma_start(xT_sb[:, cc * NT:(cc + 1) * NT],
                                 xT[cc * P:(cc + 1) * P, :])
            nc.vector.tensor_copy(x8_sb[:, cc * NT:(cc + 1) * NT],
                                  xT_sb[:, cc * NT:(cc + 1) * NT])

        def load_w(wT_dram, ncols, queues=(nc.sync,), dt=BF16):
            w_sb = wst.tile([P, CC * ncols], dt, tag="w")
            for cc in range(CC):
                queues[cc % len(queues)].dma_start(
                    w_sb[:, cc * ncols:(cc + 1) * ncols],
                    wT_dram[cc * P:(cc + 1) * P, :])
            return w_sb

        def load_w_ib(wT_dram, ib, queues=(nc.sync, nc.gpsimd)):
            """JIT [1024, 128] column-slice of a weight matrix for one i-block."""
            w_sb = wib.tile([P, CC * P], BF16, tag="wib")
            for cc in range(CC):
                queues[cc % len(queues)].dma_start(
                    w_sb[:, cc * P:(cc + 1) * P],
                    wT_dram[cc * P:(cc + 1) * P, ib * P:(ib + 1) * P])
            return w_sb

        wv_sb = load_w(wvT, CH, (nc.sync, nc.gpsimd), dt=F8)

        v3 = v_sb.rearrange("p (t s) -> p t s", t=TB)
        nc.vector.memset(v3[:, :, 8 * 65:], 0.0)
        v4 = v3[:, :, 0:8 * 65].rearrange("p t (h c) -> p t h c", c=65)
        nc.vector.memset(v4[:, :, :, 64:65], 1.0)

        x8r = x8_sb.rearrange("p (c t) -> p c t", c=CC)
        wvr = wv_sb.rearrange("p (c n) -> p c n", c=CC)

        def v_proj(tb):
            # fp8 DoubleRow: two input-channel chunks per matmul
            ps = pj.tile([P, CH], F32, tag="ps")
            for c2 in range(CC // 2):
                nc.tensor.matmul(
                    ps[:, :],
                    x8r[:, 2 * c2:2 * c2 + 2, tb * P:(tb + 1) * P],
                    wvr[:, 2 * c2:2 * c2 + 2, :],
                    start=(c2 == 0), stop=(c2 == CC // 2 - 1),
                    perf_mode=mybir.MatmulPerfMode.DoubleRow)
            vsrc = ps.rearrange("p (h c) -> p h c", c=64)
            dst = v_sb[:, tb * VS: tb * VS + 8 * 65].rearrange(
                "p (h c) -> p h c", c=65)[:, :, 0:64]
            nc.vector.tensor_copy(dst, vsrc)

        def q_proj(ib, w_sb, t_order):
            for t in t_order:
                ps = pj.tile([P, 512], F32, tag="ps")
                for cc in range(CC):
                    nc.tensor.matmul(
                        ps[:, :],
                        w_sb[:, cc * P:(cc + 1) * P],
                        xT_sb[:, cc * NT + t * 512: cc * NT + t * 512 + 512],
                        start=(cc == 0), stop=(cc == CC - 1))
                nc.vector.tensor_scalar(
                    qT_sb[:, ib * NT + t * 512: ib * NT + t * 512 + 512],
                    ps[:, :], bq_sb[:, ib:ib + 1], None, op0=ALU.add)

        def k_proj(ib, w_sb, t_order=(0, 1, 2, 3)):
            for t in t_order:
                ps = pj.tile([P, 512], F32, tag="ps")
                for cc in range(CC):
                    nc.tensor.matmul(
                        ps[:, :],
                        w_sb[:, cc * P:(cc + 1) * P],
                        xT_sb[:, cc * NT + t * 512: cc * NT + t * 512 + 512],
                        start=(cc == 0), stop=(cc == CC - 1))
                nc.vector.tensor_scalar(
                    kT_sb[:, ib * NT + t * 512: ib * NT + t * 512 + 512],
                    ps[:, :], bk_sb[:, ib:ib + 1], None, op0=ALU.add)

        # ---- prologue ----
        # Ordered so the first score chunks become ready ASAP: one kT chunk +
        # the first qT chunk, then the rest of kT, then seed v. The scheduler
        # gives earlier-emitted work higher priority, so keep this minimal.
        wq_sl = [None] * IB
        wk0 = load_w_ib(wkT, 0)
        wq_sl[0] = load_w_ib(wqT, 0)
        k_proj(0, wk0, t_order=(0,))
        q_proj(0, wq_sl[0], t_order=(2,))     # pair 0 starts on qb=2
        k_proj(0, wk0, t_order=(1, 2, 3))
        for tb in range(6):
            v_proj(tb)

        # bo broadcast to all partitions (ones matmul, once)
        for chn in range(2):
            ps = pj.tile([P, 512], F32, tag="ps")
            nc.tensor.matmul(ps[:, :], onesf[:, :], bo_sb[:, chn * 512:(chn + 1) * 512],
                             start=True, stop=True)
            nc.vector.tensor_copy(bo_bc[:, chn * 512:(chn + 1) * 512], ps[:, :])

        wo_sb = None

        def out_proj(tb, nch):
            ps = pj.tile([P, 512], F32, tag="ps")
            for j in range(IB):
                nc.tensor.matmul(
                    ps[:, :],
                    yT_sb[:, j * NT + tb * P: j * NT + (tb + 1) * P],
                    wo_sb[:, j * D + nch * 512: j * D + nch * 512 + 512],
                    start=(j == 0), stop=False)
            for j in range(IB):
                nc.tensor.matmul(
                    ps[:, :],
                    yext_sb[:, j * NO + tb * P: j * NO + (tb + 1) * P],
                    wo_sb[:, (IB + j) * D + nch * 512: (IB + j) * D + nch * 512 + 512],
                    start=False, stop=(j == IB - 1))
            os = fo.tile([P, 512], F32, tag="o")
            nc.vector.tensor_tensor(os[:, :], ps[:, :],
                                    bo_bc[:, nch * 512:(nch + 1) * 512], op=ALU.add)
            q = nc.sync if nch == 0 else nc.gpsimd
            q.dma_start(out[tb * P:(tb + 1) * P, nch * 512:(nch + 1) * 512],
                        os[:, :])

        def slot_pull(j):
            """Pull pair-j's AG slots and select the peer half into yext."""
            a = rc.tile([P, NO], BF16, tag="sel0")
            b = rc.tile([P, NO], BF16, tag="sel1")
            nc.gpsimd.dma_start(a[:, :], cc_out[2 * j * P:(2 * j + 1) * P, :])
            nc.gpsimd.dma_start(b[:, :], cc_out[(2 * j + 1) * P:(2 * j + 2) * P, :])
            nc.vector.tensor_scalar(a[:, :], a[:, :], m_sb[:, 0:1], None, op0=ALU.mult)
            nc.vector.tensor_scalar(b[:, :], b[:, :], m_sb[:, 1:2], None, op0=ALU.mult)
            nc.vector.tensor_tensor(yext_sb[:, j * NO:(j + 1) * NO], a[:, :], b[:, :],
                                    op=ALU.add)

        # ---- main attention: pr-major (4 query rounds per head pair, peer
        # token half first) so each pair's K/Q weave spreads over the whole
        # previous pair's span instead of piling into round 0 ----
        for pr in range(IB):
            for qi, qb in enumerate((2, 3, 0, 1)):
                qc = qb * 512
                hA, hB = 2 * pr, 2 * pr + 1
                oA = ao.tile([P, 512], F32, tag="oA")
                oB = ao.tile([P, 512], F32, tag="oB")
                vbA = hA * 65
                vbB = hB * 65
                for kc in range(KC):
                    s2 = sp.tile([P, 1024], F32, tag="s")
                    nc.tensor.matmul(
                        s2[:, 0:512],
                        kT_sb[0:64, pr * NT + kc * P: pr * NT + (kc + 1) * P],
                        qT_sb[0:64, pr * NT + qc: pr * NT + qc + 512],
                        start=True, stop=True)
                    nc.tensor.matmul(
                        s2[:, 512:1024],
                        kT_sb[64:128, pr * NT + kc * P: pr * NT + (kc + 1) * P],
                        qT_sb[64:128, pr * NT + qc: pr * NT + qc + 512],
                        start=True, stop=True)
                    e2 = ee.tile([P, 1024], BF16, tag="e")
                    nc.scalar.activation(e2[:, :], s2[:, :], AF.Exp, scale=SCALE)
                    nc.tensor.matmul(
                        oA[:, :],
                        v_sb[:, kc * VS + vbA: kc * VS + vbA + 128],
                        e2[:, 0:512],
                        start=(kc == 0), stop=(kc == KC - 1))
                    nc.tensor.matmul(
                        oB[:, :],
                        v_sb[:, kc * VS + vbB: kc * VS + vbB + 128],
                        e2[:, 512:1024],
                        start=(kc == 0), stop=(kc == KC - 1))
                    # first block: emit remaining v slabs a few chunks ahead
                    # of their attnV consumer (program order defines dataflow)
                    if pr == 0 and qi == 0 and kc < 10:
                        v_proj(6 + kc)
                # evict unnormalized + den rows, then normalize yT in place
                yA = yT_sb[0:64, pr * NT + qc: pr * NT + qc + 512]
                yB = yT_sb[64:128, pr * NT + qc: pr * NT + qc + 512]
                den2 = rc.tile([1, 1024], F32, tag="d")
                last = (pr == IB - 1 and qi == 3)
                nc.vector.tensor_copy(den2[0:1, 0:512], oA[64:65, :])
                nc.vector.tensor_copy(den2[0:1, 512:1024], oB[64:65, :])
                rec2 = rc.tile([1, 1024], F32, tag="rf")
                nc.vector.reciprocal_approx_fast(rec2[0:1, :], den2[0:1, :])
                nc.vector.tensor_copy(yA, oA[0:64, :])
                nc.vector.tensor_copy(yB, oB[0:64, :])
                if not last:
                    bc2 = rc.tile([P, 512], F32, tag="bc")
                    bcast_row(nc, bc2[0:64, :], rec2[0:1, 0:512], 64)
                    bcast_row(nc, bc2[64:128, :], rec2[0:1, 512:1024], 64)
                    bcA = bc2[0:64, :]
                    bcB = bc2[64:128, :]
                else:
                    # tail block: broadcast 1/den with two tiny ones-matmuls so
                    # the final out projection starts the moment yT is final
                    bpA = pj.tile([P, 512], F32, tag="ps")
                    nc.tensor.matmul(bpA[:, :], onesf[:, :], rec2[0:1, 0:512],
                                     start=True, stop=True)
                    bpB = pj.tile([P, 512], F32, tag="ps")
                    nc.tensor.matmul(bpB[:, :], onesf[:, :], rec2[0:1, 512:1024],
                                     start=True, stop=True)
                    bcA = bpA[0:64, :]
                    bcB = bpB[64:128, :]
                nc.vector.tensor_tensor(yA, yA, bcA, op=ALU.mult)
                nc.vector.tensor_scalar(yA, yA, bv_sb[0:64, pr:pr + 1], None, op0=ALU.add)
                nc.vector.tensor_tensor(yB, yB, bcB, op=ALU.mult)
                nc.vector.tensor_scalar(yB, yB, bv_sb[64:128, pr:pr + 1], None, op0=ALU.add)

                # ---- woven work (ordered by deadline, emitted low-priority
                # after each block so it fills the ACT-bound PE gaps) ----
                if pr == 0:
                    if qi == 0:
                        q_proj(0, wq_sl[0], t_order=(3, 0))
                    elif qi == 1:
                        q_proj(0, wq_sl[0], t_order=(1,))
                else:
                    if qi == 0:
                        q_proj(pr, wq_sl[pr], t_order=(0,))
                    elif qi == 1:
                        q_proj(pr, wq_sl[pr], t_order=(1,))
                if qi == 2 and pr + 1 < IB:
                    wk_next = load_w_ib(wkT, pr + 1)
                    wq_sl[pr + 1] = load_w_ib(wqT, pr + 1)
                    k_proj(pr + 1, wk_next)
                    q_proj(pr + 1, wq_sl[pr + 1], t_order=(2,))
                if qi == 3 and pr + 1 < IB:
                    q_proj(pr + 1, wq_sl[pr + 1], t_order=(3,))
                if pr == 1 and qi == 0:
                    wo_sb = load_w(woT, D, (nc.sync,))  # keep gpsimd free for the AGs
                if qi == 1:
                    # this pair's peer-token y is complete: fire its own
                    # small AllGather immediately (only the last pair's
                    # 256KB exchange lands near the tail)
                    nc.sync.dma_start(
                        cc_in[pr * P:(pr + 1) * P, :],
                        yT_sb[:, pr * NT + NO: pr * NT + NT])
                    nc.gpsimd.collective_compute(
                        "AllGather",
                        ALU.bypass,
                        replica_groups=GROUPS,
                        ins=[cc_in[pr * P:(pr + 1) * P, :]],
                        outs=[cc_out[2 * pr * P:2 * (pr + 1) * P, :]],
                    )

                if pr == IB - 1 and qi == 0:
                    # pairs 0-2 have long since landed: one pull burst here,
                    # clear of the per-block normalize chains
                    slot_pull(0)
                    slot_pull(1)
                    slot_pull(2)
                if pr == IB - 1 and qi == 2:
                    slot_pull(IB - 1)
                if pr == IB - 1 and qi == 3:
                    # token rows 0:512 (qb=0, previous round) are final --
                    # weave their output projection under the last exp span
                    for tb in range(4):
                        out_proj(tb, 0)
                        out_proj(tb, 1)

        # ---- tail: output projection for the last token rows ----
        for tb in range(4, 8):
            out_proj(tb, 0)
            out_proj(tb, 1)


N_CORES = 8

_GRAPH_CACHE = {}


def build_graph():
    if "nc" in _GRAPH_CACHE:
        return _GRAPH_CACHE["nc"]
    nc = bacc.Bacc("TRN2", target_bir_lowering=False, debug=False,
                   num_devices=N_CORES)
    xT = nc.dram_tensor("xT", [D, NT], BF16, kind="ExternalInput").ap()
    wqT = nc.dram_tensor("wqT", [D, CH], BF16, kind="ExternalInput").ap()
    wkT = nc.dram_tensor("wkT", [D, CH], BF16, kind="ExternalInput").ap()
    wvT = nc.dram_tensor("wvT", [D, CH], F8, kind="ExternalInput").ap()
    woT = nc.dram_tensor("woT", [D, D], BF16, kind="ExternalInput").ap()
    bq = nc.dram_tensor("bq", [P, IB], F32, kind="ExternalInput").ap()
    bk = nc.dram_tensor("bk", [P, IB], F32, kind="ExternalInput").ap()
    bv = nc.dram_tensor("bv", [P, IB], F32, kind="ExternalInput").ap()
    bo = nc.dram_tensor("bo", [1, D], F32, kind="ExternalInput").ap()
    msel = nc.dram_tensor("msel", [P, 2], F32, kind="ExternalInput").ap()
    out = nc.dram_tensor("out", [NO, D], F32, kind="ExternalOutput").ap()
    cc_in = nc.dram_tensor("cc_in", [CH, NO], BF16).ap()
    cc_out = nc.dram_tensor("cc_out", [2 * CH, NO], BF16).ap()
    with tile.TileContext(nc) as tc:
        attention_body(tc, out, xT, wqT, wkT, wvT, woT, bq, bk, bv, bo, msel,
                       cc_in, cc_out)
    nc.compile()
    _GRAPH_CACHE["nc"] = nc
    return nc


def make_in_maps(x, Wq, bq, Wk, bk, Wv, bv, Wo, bo):
    x = np.asarray(x, np.float32)
    wqTf = np.ascontiguousarray(np.asarray(Wq, np.float32).T)
    wkTf = np.ascontiguousarray(np.asarray(Wk, np.float32).T)
    wvTf = np.ascontiguousarray(np.asarray(Wv, np.float32).T)
    woTf = np.ascontiguousarray(np.asarray(Wo, np.float32).T)
    bqf = np.asarray(bq, np.float32)
    bkf = np.asarray(bk, np.float32)
    bvf = np.asarray(bv, np.float32)
    bof = np.asarray(bo, np.float32).reshape(1, D)
    in_maps = []
    for core in range(N_CORES):
        b, hh = core // 2, core % 2
        xb = x[b]
        if hh == 1:
            xb = np.concatenate([xb[NO:], xb[:NO]], axis=0)
        sl = slice(hh * CH, (hh + 1) * CH)
        pe = slice((1 - hh) * CH, (2 - hh) * CH)
        mvec = np.zeros((P, 2), np.float32)
        mvec[:, 1 - hh] = 1.0   # even reads slot1 (odd's contribution)
        in_maps.append({
            "xT": np.ascontiguousarray(xb.T).astype(BF),
            "wqT": np.ascontiguousarray(wqTf[:, sl]).astype(BF),
            "wkT": np.ascontiguousarray(wkTf[:, sl]).astype(BF),
            "wvT": np.ascontiguousarray(wvTf[:, sl]).astype(BF8),
            "woT": np.ascontiguousarray(
                np.concatenate([woTf[sl, :], woTf[pe, :]], axis=0)).astype(BF),
            "bq": np.ascontiguousarray(bqf[sl].reshape(IB, P).T),
            "bk": np.ascontiguousarray(bkf[sl].reshape(IB, P).T),
            "bv": np.ascontiguousarray(bvf[sl].reshape(IB, P).T),
            "bo": bof,
            "msel": mvec,
        })
    return in_maps


def run(inputs, trace=False, **kw):
    nc = build_graph()
    in_maps = make_in_maps(**inputs)
    res = run_bass_kernel_spmd(nc, in_maps, list(range(N_CORES)), trace=trace, **kw)
    x = np.asarray(inputs["x"], np.float32)
    B, N, C = x.shape
    out = np.empty((B, N, C), np.float32)
    for core in range(N_CORES):
        b, hh = core // 2, core % 2
        out[b, hh * NO:(hh + 1) * NO, :] = res.results[core]["out"]
    return out, res


def kernel(x, Wq, bq, Wk, bk, Wv, bv, Wo, bo):
    out, _ = run(dict(x=x, Wq=Wq, bq=bq, Wk=Wk, bk=bk, Wv=Wv, bv=bv, Wo=Wo, bo=bo))
    return out


# revision 5
# speedup vs baseline: 1.2163x; 1.0001x over previous
"""Multi-head attention forward (B=4, N=2048, C=1024, H=16) on 8 TRN2 NeuronCores.

Sharding v2: 8 shards = (batch b, head-half hh). Each core projects Q/K/V only
for its 8 heads (512 of 1024 channels) over all 2048 tokens of its batch --
no duplicated K/V work -- then computes attention for those heads over all
tokens. Before the output projection, the two cores of a batch exchange
attention outputs: each core sends y[peer-token-half, its 512 ch] (1MB bf16)
via a pairwise AllGather, overlapped with the attention of its own token half.
A data-driven slot select (mask input) keeps the SPMD graph core-uniform.

Same inner attention machinery as v1: bf16 TensorEngine compute, f32 PSUM,
scores transposed with per-head 64-row PE bands, exp on ScalarE (the critical
engine: 256 x ~1.15us), softmax denominator via ones-column in V, q/k bias
adds moved off ScalarE onto the DVE. Projections woven into the ACT-bound
attention blocks to keep the PE warm.
"""

from contextlib import ExitStack

import numpy as np
import ml_dtypes

import concourse.bass as bass
import concourse.bacc as bacc
import concourse.tile as tile
import concourse.mybir as mybir
from concourse.bass_utils import run_bass_kernel_spmd

F32 = mybir.dt.float32
BF16 = mybir.dt.bfloat16
AF = mybir.ActivationFunctionType
ALU = mybir.AluOpType
BF = ml_dtypes.bfloat16
F8 = mybir.dt.float8e4
BF8 = ml_dtypes.float8_e4m3

P = 128
D = 1024
CC = 8          # input-channel 128-blocks (contraction)
CH = 512        # channels (heads) per core
IB = 4          # my channel 128-blocks
NT = 2048       # tokens per batch (all in SBUF; own half first = cols 0:1024)
NO = 1024       # own tokens (output rows)
TB = 16         # 128-token key chunks
KC = 16
DH = 64
SCALE = DH ** -0.5
VS = 8 * 65 + 64  # v slab per tb: 8 heads x (64+den), padded so lhsT [128] stays in-bounds
GROUPS = [[0, 1], [2, 3], [4, 5], [6, 7]]


def bcast_row(nc, out_ap, src_row, n_part):
    """DMA-broadcast one SBUF row [1, N] to [n_part, N] via a 0-step dim."""
    ap0 = src_row.ap[0]
    free = src_row.ap[-1]
    src = bass.AP(src_row.tensor, src_row.offset, [ap0, [0, n_part], free])
    nc.sync.dma_start(out_ap, src)


def attention_body(tc, out, xT, wqT, wkT, wvT, woT, bq, bk, bv, bo, msel,
                   cc_in, cc_out):
    nc = tc.nc
    with ExitStack() as ctx:
        const = ctx.enter_context(tc.tile_pool(name="const", bufs=1))
        qkv = ctx.enter_context(tc.tile_pool(name="qkv", bufs=1))
        xw = ctx.enter_context(tc.tile_pool(name="xw", bufs=1))
        wst = ctx.enter_context(tc.tile_pool(name="wst", bufs=2))
        wib = ctx.enter_context(tc.tile_pool(name="wib", bufs=3))
        # deep e2 buffering: attnV may lag scores/exp by many chunks while the
        # PE drains woven projections -- exp must never block on a free tile
        ee = ctx.enter_context(tc.tile_pool(name="ee", bufs=8))
        rc = ctx.enter_context(tc.tile_pool(name="rc", bufs=1))
        fo = ctx.enter_context(tc.tile_pool(name="fo", bufs=2))
        sp = ctx.enter_context(tc.tile_pool(name="sp", bufs=2, space="PSUM"))
        ao = ctx.enter_context(tc.tile_pool(name="ao", bufs=1, space="PSUM"))
        pj = ctx.enter_context(tc.tile_pool(name="pj", bufs=2, space="PSUM"))

        bq_sb = const.tile([P, IB], F32)
        bk_sb = const.tile([P, IB], F32)
        bv_sb = const.tile([P, IB], F32)
        bo_sb = const.tile([1, D], F32)
        m_sb = const.tile([P, 2], F32)
        nc.scalar.dma_start(bq_sb[:, :], bq[:, :])
        nc.scalar.dma_start(bk_sb[:, :], bk[:, :])
        nc.scalar.dma_start(bv_sb[:, :], bv[:, :])
        nc.scalar.dma_start(bo_sb[:, :], bo[:, :])
        nc.scalar.dma_start(m_sb[:, :], msel[:, :])
        onesf = const.tile([1, P], F32)
        nc.vector.memset(onesf[:, :], 1.0)
        bo_bc = const.tile([P, D], BF16)

        qT_sb = qkv.tile([P, IB * NT], BF16)
        kT_sb = qkv.tile([P, IB * NT], BF16)
        v_sb = qkv.tile([P, TB * VS], BF16)
        yT_sb = qkv.tile([P, IB * NT], BF16)
        yext_sb = qkv.tile([P, IB * NO], BF16)

        # x streams on three queues so the first projection chains unblock fast
        # (ScalarE's queue is idle until the first exp, ~25us in)
        xq = [nc.sync, nc.gpsimd, nc.scalar]
        xT_sb = xw.tile([P, CC * NT], BF16)
        x8_sb = xw.tile([P, CC * NT], F8)   # fp8 shadow of x for the V matmuls
        for cc in range(CC):
            xq[cc % 3].dma_start(xT_sb[:, cc * NT:(cc + 1) * NT],
                                 xT[cc * P:(cc + 1) * P, :])
            nc.vector.tensor_copy(x8_sb[:, cc * NT:(cc + 1) * NT],
                                  xT_sb[:, cc * NT:(cc + 1) * NT])

        def load_w(wT_dram, ncols, queues=(nc.sync,), dt=BF16):
            w_sb = wst.tile([P, CC * ncols], dt, tag="w")
            for cc in range(CC):
                queues[cc % len(queues)].dma_start(
                    w_sb[:, cc * ncols:(cc + 1) * ncols],
                    wT_dram[cc * P:(cc + 1) * P, :])
            return w_sb

        def load_w_ib(wT_dram, ib, queues=(nc.sync,)):
            """JIT [1024, 128] column-slice of a weight matrix for one i-block."""
            w_sb = wib.tile([P, CC * P], BF16, tag="wib")
            for cc in range(CC):
                queues[cc % len(queues)].dma_start(
                    w_sb[:, cc * P:(cc + 1) * P],
                    wT_dram[cc * P:(cc + 1) * P, ib * P:(ib + 1) * P])
            return w_sb

        wv_sb = load_w(wvT, CH, (nc.sync, nc.gpsimd), dt=F8)

        v3 = v_sb.rearrange("p (t s) -> p t s", t=TB)
        nc.vector.memset(v3[:, :, 8 * 65:], 0.0)
        v4 = v3[:, :, 0:8 * 65].rearrange("p t (h c) -> p t h c", c=65)
        nc.vector.memset(v4[:, :, :, 64:65], 1.0)

        x8r = x8_sb.rearrange("p (c t) -> p c t", c=CC)
        wvr = wv_sb.rearrange("p (c n) -> p c n", c=CC)

        def v_proj(tb):
            # fp8 DoubleRow: two input-channel chunks per matmul
            ps = pj.tile([P, CH], F32, tag="ps")
            for c2 in range(CC // 2):
                nc.tensor.matmul(
                    ps[:, :],
                    x8r[:, 2 * c2:2 * c2 + 2, tb * P:(tb + 1) * P],
                    wvr[:, 2 * c2:2 * c2 + 2, :],
                    start=(c2 == 0), stop=(c2 == CC // 2 - 1),
                    perf_mode=mybir.MatmulPerfMode.DoubleRow)
            vsrc = ps.rearrange("p (h c) -> p h c", c=64)
            dst = v_sb[:, tb * VS: tb * VS + 8 * 65].rearrange(
                "p (h c) -> p h c", c=65)[:, :, 0:64]
            nc.vector.tensor_copy(dst, vsrc)

        def q_proj(ib, w_sb, t_order):
            for t in t_order:
                ps = pj.tile([P, 512], F32, tag="ps")
                for cc in range(CC):
                    nc.tensor.matmul(
                        ps[:, :],
                        w_sb[:, cc * P:(cc + 1) * P],
                        xT_sb[:, cc * NT + t * 512: cc * NT + t * 512 + 512],
                        start=(cc == 0), stop=(cc == CC - 1))
                nc.vector.tensor_scalar(
                    qT_sb[:, ib * NT + t * 512: ib * NT + t * 512 + 512],
                    ps[:, :], bq_sb[:, ib:ib + 1], None, op0=ALU.add)

        def k_proj(ib, w_sb, t_order=(0, 1, 2, 3)):
            for t in t_order:
                ps = pj.tile([P, 512], F32, tag="ps")
                for cc in range(CC):
                    nc.tensor.matmul(
                        ps[:, :],
                        w_sb[:, cc * P:(cc + 1) * P],
                        xT_sb[:, cc * NT + t * 512: cc * NT + t * 512 + 512],
                        start=(cc == 0), stop=(cc == CC - 1))
                nc.vector.tensor_scalar(
                    kT_sb[:, ib * NT + t * 512: ib * NT + t * 512 + 512],
                    ps[:, :], bk_sb[:, ib:ib + 1], None, op0=ALU.add)

        # ---- prologue ----
        # Ordered so the first score chunks become ready ASAP: one kT chunk +
        # the first qT chunk, then the rest of kT, then seed v. The scheduler
        # gives earlier-emitted work higher priority, so keep this minimal.
        wq_sl = [None] * IB
        wk0 = load_w_ib(wkT, 0, queues=(nc.sync, nc.gpsimd))
        wq_sl[0] = load_w_ib(wqT, 0, queues=(nc.sync, nc.gpsimd))
        k_proj(0, wk0, t_order=(0,))
        q_proj(0, wq_sl[0], t_order=(2,))     # pair 0 starts on qb=2
        k_proj(0, wk0, t_order=(1, 2, 3))
        for tb in range(6):
            v_proj(tb)

        # bo broadcast to all partitions (ones matmul, once)
        for chn in range(2):
            ps = pj.tile([P, 512], F32, tag="ps")
            nc.tensor.matmul(ps[:, :], onesf[:, :], bo_sb[:, chn * 512:(chn + 1) * 512],
                             start=True, stop=True)
            nc.vector.tensor_copy(bo_bc[:, chn * 512:(chn + 1) * 512], ps[:, :])

        wo_sb = None

        def out_proj(tb, nch):
            ps = pj.tile([P, 512], F32, tag="ps")
            for j in range(IB):
                nc.tensor.matmul(
                    ps[:, :],
                    yT_sb[:, j * NT + tb * P: j * NT + (tb + 1) * P],
                    wo_sb[:, j * D + nch * 512: j * D + nch * 512 + 512],
                    start=(j == 0), stop=False)
            for j in range(IB):
                nc.tensor.matmul(
                    ps[:, :],
                    yext_sb[:, j * NO + tb * P: j * NO + (tb + 1) * P],
                    wo_sb[:, (IB + j) * D + nch * 512: (IB + j) * D + nch * 512 + 512],
                    start=False, stop=(j == IB - 1))
            os = fo.tile([P, 512], F32, tag="o")
            nc.vector.tensor_tensor(os[:, :], ps[:, :],
                                    bo_bc[:, nch * 512:(nch + 1) * 512], op=ALU.add)
            q = nc.sync if nch == 0 else nc.gpsimd
            q.dma_start(out[tb * P:(tb + 1) * P, nch * 512:(nch + 1) * 512],
                        os[:, :])

        def slot_pull(j):
            """Pull pair-j's AG slots and select the peer half into yext."""
            a = rc.tile([P, NO], BF16, tag="sel0")
            b = rc.tile([P, NO], BF16, tag="sel1")
            nc.gpsimd.dma_start(a[:, :], cc_out[2 * j * P:(2 * j + 1) * P, :])
            nc.gpsimd.dma_start(b[:, :], cc_out[(2 * j + 1) * P:(2 * j + 2) * P, :])
            nc.vector.tensor_scalar(a[:, :], a[:, :], m_sb[:, 0:1], None, op0=ALU.mult)
            nc.vector.tensor_scalar(b[:, :], b[:, :], m_sb[:, 1:2], None, op0=ALU.mult)
            nc.vector.tensor_tensor(yext_sb[:, j * NO:(j + 1) * NO], a[:, :], b[:, :],
                                    op=ALU.add)

        # ---- main attention: pr-major (4 query rounds per head pair, peer
        # token half first) so each pair's K/Q weave spreads over the whole
        # previous pair's span instead of piling into round 0 ----
        for pr in range(IB):
            for qi, qb in enumerate((2, 3, 0, 1)):
                qc = qb * 512
                hA, hB = 2 * pr, 2 * pr + 1
                oA = ao.tile([P, 512], F32, tag="oA")
                oB = ao.tile([P, 512], F32, tag="oB")
                vbA = hA * 65
                vbB = hB * 65
                for kc in range(KC):
                    s2 = sp.tile([P, 1024], F32, tag="s")
                    nc.tensor.matmul(
                        s2[:, 0:512],
                        kT_sb[0:64, pr * NT + kc * P: pr * NT + (kc + 1) * P],
                        qT_sb[0:64, pr * NT + qc: pr * NT + qc + 512],
                        start=True, stop=True)
                    nc.tensor.matmul(
                        s2[:, 512:1024],
                        kT_sb[64:128, pr * NT + kc * P: pr * NT + (kc + 1) * P],
                        qT_sb[64:128, pr * NT + qc: pr * NT + qc + 512],
                        start=True, stop=True)
                    e2 = ee.tile([P, 1024], BF16, tag="e")
                    nc.scalar.activation(e2[:, :], s2[:, :], AF.Exp, scale=SCALE)
                    nc.tensor.matmul(
                        oA[:, :],
                        v_sb[:, kc * VS + vbA: kc * VS + vbA + 128],
                        e2[:, 0:512],
                        start=(kc == 0), stop=(kc == KC - 1))
                    nc.tensor.matmul(
                        oB[:, :],
                        v_sb[:, kc * VS + vbB: kc * VS + vbB + 128],
                        e2[:, 512:1024],
                        start=(kc == 0), stop=(kc == KC - 1))
                    # first block: emit remaining v slabs a few chunks ahead
                    # of their attnV consumer (program order defines dataflow)
                    if pr == 0 and qi == 0 and kc < 10:
                        v_proj(6 + kc)
                # evict unnormalized + den rows, then normalize yT in place
                yA = yT_sb[0:64, pr * NT + qc: pr * NT + qc + 512]
                yB = yT_sb[64:128, pr * NT + qc: pr * NT + qc + 512]
                den2 = rc.tile([1, 1024], F32, tag="d")
                last = (pr == IB - 1 and qi == 3)
                nc.vector.tensor_copy(den2[0:1, 0:512], oA[64:65, :])
                nc.vector.tensor_copy(den2[0:1, 512:1024], oB[64:65, :])
                rec2 = rc.tile([1, 1024], F32, tag="rf")
                nc.vector.reciprocal_approx_fast(rec2[0:1, :], den2[0:1, :])
                nc.vector.tensor_copy(yA, oA[0:64, :])
                nc.vector.tensor_copy(yB, oB[0:64, :])
                if not last:
                    bc2 = rc.tile([P, 512], F32, tag="bc")
                    bcast_row(nc, bc2[0:64, :], rec2[0:1, 0:512], 64)
                    bcast_row(nc, bc2[64:128, :], rec2[0:1, 512:1024], 64)
                    bcA = bc2[0:64, :]
                    bcB = bc2[64:128, :]
                else:
                    # tail block: broadcast 1/den with two tiny ones-matmuls so
                    # the final out projection starts the moment yT is final
                    bpA = pj.tile([P, 512], F32, tag="ps")
                    nc.tensor.matmul(bpA[:, :], onesf[:, :], rec2[0:1, 0:512],
                                     start=True, stop=True)
                    bpB = pj.tile([P, 512], F32, tag="ps")
                    nc.tensor.matmul(bpB[:, :], onesf[:, :], rec2[0:1, 512:1024],
                                     start=True, stop=True)
                    bcA = bpA[0:64, :]
                    bcB = bpB[64:128, :]
                nc.vector.tensor_tensor(yA, yA, bcA, op=ALU.mult)
                nc.vector.tensor_scalar(yA, yA, bv_sb[0:64, pr:pr + 1], None, op0=ALU.add)
                nc.vector.tensor_tensor(yB, yB, bcB, op=ALU.mult)
                nc.vector.tensor_scalar(yB, yB, bv_sb[64:128, pr:pr + 1], None, op0=ALU.add)

                # ---- woven work (ordered by deadline, emitted low-priority
                # after each block so it fills the ACT-bound PE gaps) ----
                if pr == 0:
                    if qi == 0:
                        q_proj(0, wq_sl[0], t_order=(3, 0))
                    elif qi == 1:
                        q_proj(0, wq_sl[0], t_order=(1,))
                else:
                    if qi == 0:
                        q_proj(pr, wq_sl[pr], t_order=(0,))
                    elif qi == 1:
                        q_proj(pr, wq_sl[pr], t_order=(1,))
                if qi == 2 and pr + 1 < IB:
                    wk_next = load_w_ib(wkT, pr + 1)
                    wq_sl[pr + 1] = load_w_ib(wqT, pr + 1)
                    k_proj(pr + 1, wk_next)
                    q_proj(pr + 1, wq_sl[pr + 1], t_order=(2,))
                if qi == 3 and pr + 1 < IB:
                    q_proj(pr + 1, wq_sl[pr + 1], t_order=(3,))
                if pr == 1 and qi == 0:
                    wo_sb = load_w(woT, D, (nc.sync,))  # keep gpsimd free for the AGs
                if qi == 1:
                    # this pair's peer-token y is complete: fire its own
                    # small AllGather immediately (only the last pair's
                    # 256KB exchange lands near the tail)
                    nc.gpsimd.dma_start(
                        cc_in[pr * P:(pr + 1) * P, :],
                        yT_sb[:, pr * NT + NO: pr * NT + NT])
                    nc.gpsimd.collective_compute(
                        "AllGather",
                        ALU.bypass,
                        replica_groups=GROUPS,
                        ins=[cc_in[pr * P:(pr + 1) * P, :]],
                        outs=[cc_out[2 * pr * P:2 * (pr + 1) * P, :]],
                    )

                if pr == IB - 1 and qi == 0:
                    # pairs 0-2 have long since landed: one pull burst here,
                    # clear of the per-block normalize chains
                    slot_pull(0)
                    slot_pull(1)
                    slot_pull(2)
                if pr == IB - 1 and qi == 2:
                    slot_pull(IB - 1)
                if pr == IB - 1 and qi == 3:
                    # token rows 0:512 (qb=0, previous round) are final --
                    # weave their output projection under the last exp span
                    for tb in range(4):
                        out_proj(tb, 0)
                        out_proj(tb, 1)

        # ---- tail: output projection for the last token rows ----
        for tb in range(4, 8):
            out_proj(tb, 0)
            out_proj(tb, 1)


N_CORES = 8

_GRAPH_CACHE = {}


def build_graph():
    if "nc" in _GRAPH_CACHE:
        return _GRAPH_CACHE["nc"]
    nc = bacc.Bacc("TRN2", target_bir_lowering=False, debug=False,
                   num_devices=N_CORES)
    xT = nc.dram_tensor("xT", [D, NT], BF16, kind="ExternalInput").ap()
    wqT = nc.dram_tensor("wqT", [D, CH], BF16, kind="ExternalInput").ap()
    wkT = nc.dram_tensor("wkT", [D, CH], BF16, kind="ExternalInput").ap()
    wvT = nc.dram_tensor("wvT", [D, CH], F8, kind="ExternalInput").ap()
    woT = nc.dram_tensor("woT", [D, D], BF16, kind="ExternalInput").ap()
    bq = nc.dram_tensor("bq", [P, IB], F32, kind="ExternalInput").ap()
    bk = nc.dram_tensor("bk", [P, IB], F32, kind="ExternalInput").ap()
    bv = nc.dram_tensor("bv", [P, IB], F32, kind="ExternalInput").ap()
    bo = nc.dram_tensor("bo", [1, D], F32, kind="ExternalInput").ap()
    msel = nc.dram_tensor("msel", [P, 2], F32, kind="ExternalInput").ap()
    out = nc.dram_tensor("out", [NO, D], F32, kind="ExternalOutput").ap()
    cc_in = nc.dram_tensor("cc_in", [CH, NO], BF16).ap()
    cc_out = nc.dram_tensor("cc_out", [2 * CH, NO], BF16).ap()
    with tile.TileContext(nc) as tc:
        attention_body(tc, out, xT, wqT, wkT, wvT, woT, bq, bk, bv, bo, msel,
                       cc_in, cc_out)
    nc.compile()
    _GRAPH_CACHE["nc"] = nc
    return nc


def make_in_maps(x, Wq, bq, Wk, bk, Wv, bv, Wo, bo):
    x = np.asarray(x, np.float32)
    wqTf = np.ascontiguousarray(np.asarray(Wq, np.float32).T)
    wkTf = np.ascontiguousarray(np.asarray(Wk, np.float32).T)
    wvTf = np.ascontiguousarray(np.asarray(Wv, np.float32).T)
    woTf = np.ascontiguousarray(np.asarray(Wo, np.float32).T)
    bqf = np.asarray(bq, np.float32)
    bkf = np.asarray(bk, np.float32)
    bvf = np.asarray(bv, np.float32)
    bof = np.asarray(bo, np.float32).reshape(1, D)
    in_maps = []
    for core in range(N_CORES):
        b, hh = core // 2, core % 2
        xb = x[b]
        if hh == 1:
            xb = np.concatenate([xb[NO:], xb[:NO]], axis=0)
        sl = slice(hh * CH, (hh + 1) * CH)
        pe = slice((1 - hh) * CH, (2 - hh) * CH)
        mvec = np.zeros((P, 2), np.float32)
        mvec[:, 1 - hh] = 1.0   # even reads slot1 (odd's contribution)
        in_maps.append({
            "xT": np.ascontiguousarray(xb.T).astype(BF),
            "wqT": np.ascontiguousarray(wqTf[:, sl]).astype(BF),
            "wkT": np.ascontiguousarray(wkTf[:, sl]).astype(BF),
            "wvT": np.ascontiguousarray(wvTf[:, sl]).astype(BF8),
            "woT": np.ascontiguousarray(
                np.concatenate([woTf[sl, :], woTf[pe, :]], axis=0)).astype(BF),
            "bq": np.ascontiguousarray(bqf[sl].reshape(IB, P).T),
            "bk": np.ascontiguousarray(bkf[sl].reshape(IB, P).T),
            "bv": np.ascontiguousarray(bvf[sl].reshape(IB, P).T),
            "bo": bof,
            "msel": mvec,
        })
    return in_maps


def run(inputs, trace=False, **kw):
    nc = build_graph()
    in_maps = make_in_maps(**inputs)
    res = run_bass_kernel_spmd(nc, in_maps, list(range(N_CORES)), trace=trace, **kw)
    x = np.asarray(inputs["x"], np.float32)
    B, N, C = x.shape
    out = np.empty((B, N, C), np.float32)
    for core in range(N_CORES):
        b, hh = core // 2, core % 2
        out[b, hh * NO:(hh + 1) * NO, :] = res.results[core]["out"]
    return out, res


def kernel(x, Wq, bq, Wk, bk, Wv, bv, Wo, bo):
    out, _ = run(dict(x=x, Wq=Wq, bq=bq, Wk=Wk, bk=bk, Wv=Wv, bv=bv, Wo=Wo, bo=bo))
    return out
